# revision 8
# baseline (speedup 1.0000x reference)
"""Trainium2 Bass kernel for nn_Circuit_19275813225041.

24-qubit state-vector simulation: one layer of single-qubit gates on every
qubit, then a ladder of two-qubit gates on neighboring pairs (q, q+1),
q = 0..22, on a 2^24 complex state stored as (2, 2^24) float32 (re, im).

Strategy (8 NeuronCores):
  - Qubit q <-> bit q of the state index, bit 0 = MSB.
  - Shard the state over the 3 LSB qubits (q21,q22,q23): core d holds
    amplitudes with index % 8 == d (state-vector slicing).
  - Gates are fused on the host into 4 big chunk matrices:
      U1: 128x128 on qubits [0..6]    (singles 0..6, ladder (0,1)..(5,6))
      U2: 256x256 on qubits [6..13]   (singles 7..13, ladder (6,7)..(12,13))
      U3: 256x256 on qubits [13..20]  (singles 14..20, ladder (13,14)..(19,20))
      U4: 128x128 on qubits [21,22,23,17..20] (singles 21..23, ladder
          (20,21),(21,22),(22,23), identity on q17..q19)
    (The reference's _apply_gate has a permutation quirk for the 2-qubit
    gate at q=1 -- its "inverse" transpose applies perm again, which for
    q=1 is a 3-cycle.  This adds a relabeling permutation on qubits
    (0,1,2) right after that gate; it is folded into U1.)
  - Each core applies U1..U3 to its local 2^21 state via TensorE matmuls,
    with PE transposes rotating 7-bit groups through the partition axis and
    2-term PSUM accumulation handling the chunk boundary bit (q6, q13).
  - One AllToAll swaps qubits (q0,q1,q2) <-> (q21,q22,q23) across cores so
    the final chunk U4 applies locally; output is returned sharded over
    (q0,q1,q2) and reassembled on the host.
"""

import numpy as np

import concourse.bass as bass
import concourse.bacc as bacc
import concourse.mybir as mybir
import concourse.tile as tile
from concourse.bass_utils import run_bass_kernel_spmd

F32 = mybir.dt.float32
F32R = mybir.dt.float32r

USE_F32R = False  # fast fp32 matmul mode (reduced precision); validated below
N_CORES = 8


# ---------------------------------------------------------------------------
# Host-side gate fusion
# ---------------------------------------------------------------------------

def _embed_gate(mat, qubits, group):
    """Embed `mat` acting on `qubits` (MSB-first) into the space indexed by
    `group` (list of qubits, group[0] = MSB of the index)."""
    g = len(group)
    k = len(qubits)
    pos = [group.index(q) for q in qubits]
    rest = [i for i in range(g) if i not in pos]
    U = np.zeros((1 << g, 1 << g), dtype=np.complex128)
    for r in range(1 << len(rest)):
        base = 0
        for bi, p in enumerate(rest):
            if (r >> (len(rest) - 1 - bi)) & 1:
                base |= 1 << (g - 1 - p)
        for a in range(1 << k):
            ia = base
            for bi, p in enumerate(pos):
                if (a >> (k - 1 - bi)) & 1:
                    ia |= 1 << (g - 1 - p)
            for b in range(1 << k):
                ib = base
                for bi, p in enumerate(pos):
                    if (b >> (k - 1 - bi)) & 1:
                        ib |= 1 << (g - 1 - p)
                U[ia, ib] = mat[a, b]
    return U


def _quirk_P():
    # reference._apply_gate on [1,2]: the un-permute uses perm (a 3-cycle)
    # instead of its inverse => extra relabeling on qubits (0,1,2):
    # new (b0,b1,b2) = (old b2, old b0, old b1).
    P = np.zeros((8, 8), dtype=np.complex128)
    for b0 in range(2):
        for b1 in range(2):
            for b2 in range(2):
                P[(b2 << 2) | (b0 << 1) | b1, (b0 << 2) | (b1 << 1) | b2] = 1
    return P


def _fuse(ops, group):
    U = np.eye(1 << len(group), dtype=np.complex128)
    for mat, qb in ops:
        U = _embed_gate(mat, qb, group) @ U
    return U


def build_chunk_matrices(gates1, gates2):
    g1 = gates1[:, 0].astype(np.float64) + 1j * gates1[:, 1].astype(np.float64)
    g2 = gates2[:, 0].astype(np.float64) + 1j * gates2[:, 1].astype(np.float64)

    ops1 = [(g1[q], [q]) for q in range(0, 7)]
    ops1 += [(g2[0], [0, 1]), (g2[1], [1, 2]), (_quirk_P(), [0, 1, 2])]
    ops1 += [(g2[q], [q, q + 1]) for q in range(2, 6)]
    U1 = _fuse(ops1, list(range(0, 7)))

    ops2 = [(g1[q], [q]) for q in range(7, 14)]
    ops2 += [(g2[q], [q, q + 1]) for q in range(6, 13)]
    U2 = _fuse(ops2, list(range(6, 14)))  # q6 = MSB of the 256 index

    ops3 = [(g1[q], [q]) for q in range(14, 21)]
    ops3 += [(g2[q], [q, q + 1]) for q in range(13, 20)]
    U3 = _fuse(ops3, list(range(13, 21)))  # q13 = MSB

    ops4 = [(g1[q], [q]) for q in range(21, 24)]
    ops4 += [(g2[q], [q, q + 1]) for q in range(20, 23)]
    # partition index on the device = s*16 + m, s = (q21,q22,q23), m = (q17..q20)
    U4 = _fuse(ops4, [21, 22, 23, 17, 18, 19, 20])

    return U1, U2, U3, U4


def _pack_lhsT(U):
    """lhsT components for out = U @ x (complex):  A = re(U)^T, B = im(U)^T,
    Bn = -im(U)^T, stacked (3, n, n) float32."""
    return np.stack([U.real.T, U.imag.T, -U.imag.T]).astype(np.float32)


def build_weights(gates1, gates2):
    U1, U2, U3, U4 = build_chunk_matrices(gates1, gates2)
    w1 = _pack_lhsT(U1)
    w4 = _pack_lhsT(U4)

    def blocks(U):  # (2, 2, 3, 128, 128)
        return np.stack([
            np.stack([_pack_lhsT(U[j * 128:(j + 1) * 128, k * 128:(k + 1) * 128])
                      for k in (0, 1)])
            for j in (0, 1)])

    w2 = blocks(U2)
    w3 = blocks(U3)
    ident = np.eye(128, dtype=np.float32)
    return {"w1": w1, "w2": w2, "w3": w3, "w4": w4, "ident": ident}


# ---------------------------------------------------------------------------
# Bass kernel builder
# ---------------------------------------------------------------------------

def build_nc(use_f32r=USE_F32R):
    nc = bacc.Bacc()

    st = nc.declare_dram_parameter("state", [2, 1 << 21], F32, isOutput=False)
    w1 = nc.declare_dram_parameter("w1", [3, 128, 128], F32, isOutput=False)
    w2 = nc.declare_dram_parameter("w2", [2, 2, 3, 128, 128], F32, isOutput=False)
    w3 = nc.declare_dram_parameter("w3", [2, 2, 3, 128, 128], F32, isOutput=False)
    w4 = nc.declare_dram_parameter("w4", [3, 128, 128], F32, isOutput=False)
    idn = nc.declare_dram_parameter("ident", [128, 128], F32, isOutput=False)
    out = nc.declare_dram_parameter("out", [2, 1 << 21], F32, isOutput=True)

    # AllToAll bounce buffers: [block(dest/src rank), plane, part, inner]
    a2a_in = nc.dram_tensor("a2a_in", [8, 2, 128, 2048], F32)
    a2a_out = nc.dram_tensor("a2a_out", [8, 2, 128, 2048], F32)

    def rr(ap):
        return ap.bitcast(F32R) if use_f32r else ap

    with tile.TileContext(nc, num_cores=N_CORES) as tc:
        with tc.tile_pool(name="state", bufs=1) as sp, \
             tc.tile_pool(name="wpool", bufs=1) as wp, \
             tc.tile_pool(name="mm", bufs=6, space="PSUM") as mmp, \
             tc.tile_pool(name="tr", bufs=2, space="PSUM") as trp:

            sre = sp.tile([128, 16384], F32, tag="sre")
            sim = sp.tile([128, 16384], F32, tag="sim")

            # ---- load weights ----
            def load_w3(dram_ap3, name):  # (3,128,128) -> 3 sbuf tiles
                ts = []
                for i in range(3):
                    t = wp.tile([128, 128], F32, tag=f"{name}_{i}")
                    nc.sync.dma_start(out=t[:], in_=dram_ap3[i])
                    ts.append(t)
                return ts

            w1t = load_w3(w1, "w1")
            w4t = load_w3(w4, "w4")
            w2t = [[load_w3(w2[j, k], f"w2_{j}{k}") for k in (0, 1)] for j in (0, 1)]
            w3t = [[load_w3(w3[j, k], f"w3_{j}{k}") for k in (0, 1)] for j in (0, 1)]
            idt = wp.tile([128, 128], F32, tag="ident")
            nc.sync.dma_start(out=idt[:], in_=idn[:])

            # ---- load state:  partitions (q0..q6), free (q7..q20) ----
            for pl, s in ((0, sre), (1, sim)):
                nc.sync.dma_start(
                    out=s[:], in_=st[pl].rearrange("(p f) -> p f", p=128))

            planes = ((sre, sim))

            def cmul_into(pre, pim, W, xre, xim, start, stop=False):
                """pre += re(U)@xre - im(U)@xim ; pim += im(U)@xre + re(U)@xim
                W = [A, B, Bn] lhsT tiles."""
                A, B, Bn = W
                nc.tensor.matmul(pre[:], rr(A[:]), rr(xre), start=start, stop=False)
                nc.tensor.matmul(pre[:], rr(Bn[:]), rr(xim), start=False, stop=stop)
                nc.tensor.matmul(pim[:], rr(B[:]), rr(xre), start=start, stop=False)
                nc.tensor.matmul(pim[:], rr(A[:]), rr(xim), start=False, stop=stop)

            # ---- P1: chunk on partitions (q0..q6) ----
            for t in range(32):
                c0 = t * 512
                pre = mmp.tile([128, 512], F32, tag="mm")
                pim = mmp.tile([128, 512], F32, tag="mm")
                xre = sre[:, c0:c0 + 512]
                xim = sim[:, c0:c0 + 512]
                A, B, Bn = w1t
                nc.tensor.matmul(pre[:], rr(A[:]), rr(xre), start=True, stop=False)
                nc.tensor.matmul(pre[:], rr(Bn[:]), rr(xim), start=False, stop=True)
                nc.tensor.matmul(pim[:], rr(B[:]), rr(xre), start=True, stop=False)
                nc.tensor.matmul(pim[:], rr(A[:]), rr(xim), start=False, stop=True)
                nc.vector.tensor_copy(sre[:, c0:c0 + 512], pre[:])
                nc.scalar.copy(out=sim[:, c0:c0 + 512], in_=pim[:])

            # ---- T1: transpose partitions (q0..q6) <-> free (q7..q13) ----
            # L1 free = (q7..q13)*128 + (q14..q20); window w = (q14..q20):
            # read col-set {a*128+w}, transpose, write back to same col-set,
            # giving L2: partitions (q7..q13), free = (q0..q6)*128 + (q14..q20).
            for s in (sre, sim):
                sv = s[:].rearrange("p (a w) -> p a w", w=128)
                for w in range(128):
                    pt = trp.tile([128, 128], F32, tag="tr")
                    nc.tensor.transpose(pt[:], rr(sv[:, :, w]), rr(idt[:]))
                    nc.vector.tensor_copy(sv[:, :, w], pt[:])

            # ---- P2: chunk [6..13]; partitions (q7..q13), q6 = free bit ----
            # L2 free = (q0..q6)*128 + (q14..q20); q6 = bit0 of the outer
            # index => columns alternate 128-blocks by q6.
            sre_v = sre[:].rearrange("p (o q c) -> p o q c", q=2, c=128)
            sim_v = sim[:].rearrange("p (o q c) -> p o q c", q=2, c=128)
            for t in range(16):
                o0 = t * 4
                xr = [sre_v[:, o0:o0 + 4, k, :] for k in (0, 1)]
                xi = [sim_v[:, o0:o0 + 4, k, :] for k in (0, 1)]
                ps = []
                for j in (0, 1):
                    pre = mmp.tile([128, 512], F32, tag="mm")
                    pim = mmp.tile([128, 512], F32, tag="mm")
                    cmul_into(pre, pim, w2t[j][0], xr[0], xi[0], start=True)
                    cmul_into(pre, pim, w2t[j][1], xr[1], xi[1], start=False, stop=True)
                    ps.append((pre, pim))
                for j in (0, 1):
                    pre, pim = ps[j]
                    nc.vector.tensor_copy(sre_v[:, o0:o0 + 4, j, :], pre[:])
                    nc.scalar.copy(out=sim_v[:, o0:o0 + 4, j, :], in_=pim[:])

            # ---- T2: transpose partitions (q7..q13) <-> free (q14..q20) ----
            # window o = (q0..q6): read contiguous block [o*128, o*128+128),
            # write back contiguous, giving L3: partitions (q14..q20),
            # free = (q0..q6)*128 + (q7..q13) = (q0..q13) natural.
            for s in (sre, sim):
                for o in range(128):
                    pt = trp.tile([128, 128], F32, tag="tr")
                    nc.tensor.transpose(
                        pt[:], rr(s[:, o * 128:o * 128 + 128]), rr(idt[:]))
                    nc.vector.tensor_copy(s[:, o * 128:o * 128 + 128], pt[:])

            # ---- P3: chunk [13..20]; partitions (q14..q20), q13 = free bit0 ----
            sre_w = sre[:].rearrange("p (c k) -> p c k", k=2)
            sim_w = sim[:].rearrange("p (c k) -> p c k", k=2)
            for t in range(16):
                c0 = t * 512
                xr = [sre_w[:, c0:c0 + 512, k] for k in (0, 1)]
                xi = [sim_w[:, c0:c0 + 512, k] for k in (0, 1)]
                ps = []
                for j in (0, 1):
                    pre = mmp.tile([128, 512], F32, tag="mm")
                    pim = mmp.tile([128, 512], F32, tag="mm")
                    cmul_into(pre, pim, w3t[j][0], xr[0], xi[0], start=True)
                    cmul_into(pre, pim, w3t[j][1], xr[1], xi[1], start=False, stop=True)
                    ps.append((pre, pim))
                for j in (0, 1):
                    pre, pim = ps[j]
                    nc.vector.tensor_copy(sre_w[:, c0:c0 + 512, j], pre[:])
                    nc.scalar.copy(out=sim_w[:, c0:c0 + 512, j], in_=pim[:])

            # ---- A2A staging:  SBUF (part q14..q20, free q0..q13) ->
            #      a2a_in[b = (q0,q1,q2), plane, part, (q3..q13)] ----
            for pl, s in ((0, sre), (1, sim)):
                nc.sync.dma_start(
                    out=a2a_in[:, pl].rearrange("b p f -> p b f"),
                    in_=s[:].rearrange("p (b f) -> p b f", b=8))

            nc.gpsimd.collective_compute(
                "AllToAll",
                mybir.AluOpType.bypass,
                replica_groups=[list(range(N_CORES))],
                ins=[a2a_in.ap().opt()],
                outs=[a2a_out.ap().opt()],
            )

            # ---- P4 readback: a2a_out[s3, pl, (h,m), f] ->
            #      partitions (s3,m) = s3*16+m, free = h*2048 + f,
            #      h = (q14,q15,q16), m = (q17..q20), f = (q3..q13) ----
            for s3 in range(8):
                for pl, s in ((0, sre), (1, sim)):
                    nc.sync.dma_start(
                        out=s[s3 * 16:(s3 + 1) * 16, :]
                            .rearrange("m (h f) -> m h f", h=8),
                        in_=a2a_out[s3, pl].rearrange("(h m) f -> m h f", m=16))

            # ---- P4: chunk [20..23] on partitions (q21,q22,q23,q17..q20) ----
            for t in range(32):
                c0 = t * 512
                pre = mmp.tile([128, 512], F32, tag="mm")
                pim = mmp.tile([128, 512], F32, tag="mm")
                xre = sre[:, c0:c0 + 512]
                xim = sim[:, c0:c0 + 512]
                A, B, Bn = w4t
                nc.tensor.matmul(pre[:], rr(A[:]), rr(xre), start=True, stop=False)
                nc.tensor.matmul(pre[:], rr(Bn[:]), rr(xim), start=False, stop=True)
                nc.tensor.matmul(pim[:], rr(B[:]), rr(xre), start=True, stop=False)
                nc.tensor.matmul(pim[:], rr(A[:]), rr(xim), start=False, stop=True)
                nc.vector.tensor_copy(sre[:, c0:c0 + 512], pre[:])
                nc.scalar.copy(out=sim[:, c0:c0 + 512], in_=pim[:])

            # ---- store:  out[pl] = partition-major flat ----
            for pl, s in ((0, sre), (1, sim)):
                nc.sync.dma_start(
                    out=out[pl].rearrange("(p f) -> p f", p=128), in_=s[:])

    return nc


# ---------------------------------------------------------------------------
# Host wrapper
# ---------------------------------------------------------------------------

TRACE = False          # set by test harnesses to capture a profile
LAST_EXEC_NS = None
LAST_RESULTS = None


def kernel(state, gates1, gates2):
    global LAST_EXEC_NS, LAST_RESULTS
    state = np.ascontiguousarray(np.asarray(state, dtype=np.float32))
    weights = build_weights(np.asarray(gates1, dtype=np.float32),
                            np.asarray(gates2, dtype=np.float32))

    # shard over (q21,q22,q23) = index mod 8
    shards = np.ascontiguousarray(
        state.reshape(2, 1 << 21, 8).transpose(2, 0, 1))

    nc = build_nc()
    if not nc.is_finalized():
        nc.finalize()
    in_maps = [dict(weights, state=shards[d]) for d in range(N_CORES)]
    res = run_bass_kernel_spmd(nc, in_maps, core_ids=list(range(N_CORES)),
                               trace=TRACE)
    LAST_EXEC_NS = res.exec_time_ns
    LAST_RESULTS = res

    # unshard: core d holds (q0,q1,q2) = d;
    # out layout = [plane][s=(q21..q23), m=(q17..q20)][h=(q14..q16)][f=(q3..q13)]
    full = np.empty((2, 8, 2048, 8, 16, 8), dtype=np.float32)
    for d in range(N_CORES):
        od = res.results[d]["out"].reshape(2, 8, 16, 8, 2048)
        full[:, d] = od.transpose(0, 4, 3, 2, 1)
    return full.reshape(2, 1 << 24)


if __name__ == "__main__":
    rng = np.random.default_rng(0)
    state = rng.standard_normal((2, 1 << 24)).astype(np.float32)
    g1 = rng.standard_normal((24, 2, 2, 2)).astype(np.float32)
    g2 = rng.standard_normal((23, 2, 4, 4)).astype(np.float32)
    out = kernel(state, g1, g2)
    print(out.shape, out.dtype)


# revision 11
# speedup vs baseline: 1.4906x; 1.4906x over previous
"""Trainium2 Bass kernel for nn_Circuit_19275813225041.

24-qubit state-vector simulation: one layer of single-qubit gates on every
qubit, then a ladder of two-qubit gates on neighboring pairs (q, q+1),
q = 0..22, on a 2^24 complex state stored as (2, 2^24) float32 (re, im).

Strategy (8 NeuronCores):
  - Qubit q <-> bit q of the state index, bit 0 = MSB.
  - Shard the state over the 3 LSB qubits (q21,q22,q23): core d holds
    amplitudes with index % 8 == d (state-vector slicing).
  - Gates are fused on the host into 4 big chunk matrices:
      U1: 128x128 on qubits [0..6]    (singles 0..6, ladder (0,1)..(5,6))
      U2: 256x256 on qubits [6..13]   (singles 7..13, ladder (6,7)..(12,13))
      U3: 256x256 on qubits [13..20]  (singles 14..20, ladder (13,14)..(19,20))
      U4: 128x128 on qubits [21,22,23,17..20] (singles 21..23, ladder
          (20,21),(21,22),(22,23), identity on q17..q19)
    (The reference's _apply_gate has a permutation quirk for the 2-qubit
    gate at q=1 -- its "inverse" transpose applies perm again, which for
    q=1 is a 3-cycle.  This adds a relabeling permutation on qubits
    (0,1,2) right after that gate; it is folded into U1.)
  - Each core applies U1..U3 to its local 2^21 state via TensorE matmuls,
    with PE transposes rotating 7-bit groups through the partition axis and
    2-term PSUM accumulation handling the chunk boundary bit (q6, q13).
  - One AllToAll swaps qubits (q0,q1,q2) <-> (q21,q22,q23) across cores so
    the final chunk U4 applies locally; output is returned sharded over
    (q0,q1,q2) and reassembled on the host.
"""

import numpy as np

import concourse.bass as bass
import concourse.bacc as bacc
import concourse.mybir as mybir
import concourse.tile as tile
from concourse.bass_utils import run_bass_kernel_spmd

F32 = mybir.dt.float32
F32R = mybir.dt.float32r

USE_F32R = True   # fast fp32 matmul mode (reduced precision)
N_CORES = 8


# ---------------------------------------------------------------------------
# Host-side gate fusion
# ---------------------------------------------------------------------------

def _embed_gate(mat, qubits, group):
    """Embed `mat` acting on `qubits` (MSB-first) into the space indexed by
    `group` (list of qubits, group[0] = MSB of the index)."""
    g = len(group)
    k = len(qubits)
    pos = [group.index(q) for q in qubits]
    rest = [i for i in range(g) if i not in pos]
    U = np.zeros((1 << g, 1 << g), dtype=np.complex128)
    for r in range(1 << len(rest)):
        base = 0
        for bi, p in enumerate(rest):
            if (r >> (len(rest) - 1 - bi)) & 1:
                base |= 1 << (g - 1 - p)
        for a in range(1 << k):
            ia = base
            for bi, p in enumerate(pos):
                if (a >> (k - 1 - bi)) & 1:
                    ia |= 1 << (g - 1 - p)
            for b in range(1 << k):
                ib = base
                for bi, p in enumerate(pos):
                    if (b >> (k - 1 - bi)) & 1:
                        ib |= 1 << (g - 1 - p)
                U[ia, ib] = mat[a, b]
    return U


def _quirk_P():
    # reference._apply_gate on [1,2]: the un-permute uses perm (a 3-cycle)
    # instead of its inverse => extra relabeling on qubits (0,1,2):
    # new (b0,b1,b2) = (old b2, old b0, old b1).
    P = np.zeros((8, 8), dtype=np.complex128)
    for b0 in range(2):
        for b1 in range(2):
            for b2 in range(2):
                P[(b2 << 2) | (b0 << 1) | b1, (b0 << 2) | (b1 << 1) | b2] = 1
    return P


def _fuse(ops, group):
    U = np.eye(1 << len(group), dtype=np.complex128)
    for mat, qb in ops:
        U = _embed_gate(mat, qb, group) @ U
    return U


def build_chunk_matrices(gates1, gates2):
    g1 = gates1[:, 0].astype(np.float64) + 1j * gates1[:, 1].astype(np.float64)
    g2 = gates2[:, 0].astype(np.float64) + 1j * gates2[:, 1].astype(np.float64)

    ops1 = [(g1[q], [q]) for q in range(0, 7)]
    ops1 += [(g2[0], [0, 1]), (g2[1], [1, 2]), (_quirk_P(), [0, 1, 2])]
    ops1 += [(g2[q], [q, q + 1]) for q in range(2, 6)]
    U1 = _fuse(ops1, list(range(0, 7)))

    ops2 = [(g1[q], [q]) for q in range(7, 14)]
    ops2 += [(g2[q], [q, q + 1]) for q in range(6, 13)]
    U2 = _fuse(ops2, list(range(6, 14)))  # q6 = MSB of the 256 index

    ops3 = [(g1[q], [q]) for q in range(14, 21)]
    ops3 += [(g2[q], [q, q + 1]) for q in range(13, 20)]
    U3 = _fuse(ops3, list(range(13, 21)))  # q13 = MSB

    ops4 = [(g1[q], [q]) for q in range(21, 24)]
    ops4 += [(g2[q], [q, q + 1]) for q in range(20, 23)]
    # partition index on the device = s*16 + m, s = (q21,q22,q23), m = (q17..q20)
    U4 = _fuse(ops4, [21, 22, 23, 17, 18, 19, 20])

    return U1, U2, U3, U4


def _pack_lhsT(U):
    """lhsT components for out = U @ x (complex):  A = re(U)^T, B = im(U)^T,
    Bn = -im(U)^T, stacked (3, n, n) float32."""
    return np.stack([U.real.T, U.imag.T, -U.imag.T]).astype(np.float32)


def build_weights(gates1, gates2):
    U1, U2, U3, U4 = build_chunk_matrices(gates1, gates2)
    w1 = _pack_lhsT(U1)
    w4 = _pack_lhsT(U4)

    def blocks(U):  # (2, 2, 3, 128, 128)
        return np.stack([
            np.stack([_pack_lhsT(U[j * 128:(j + 1) * 128, k * 128:(k + 1) * 128])
                      for k in (0, 1)])
            for j in (0, 1)])

    w2 = blocks(U2)
    w3 = blocks(U3)
    ident = np.eye(128, dtype=np.float32)
    return {"w1": w1, "w2": w2, "w3": w3, "w4": w4, "ident": ident}


# ---------------------------------------------------------------------------
# Bass kernel builder
# ---------------------------------------------------------------------------

def build_nc(use_f32r=USE_F32R):
    nc = bacc.Bacc()
    SD_DRAM = F32R if use_f32r else F32

    st = nc.declare_dram_parameter("state", [2, 1 << 21], F32, isOutput=False)
    w1 = nc.declare_dram_parameter("w1", [3, 128, 128], F32, isOutput=False)
    w2 = nc.declare_dram_parameter("w2", [2, 2, 3, 128, 128], F32, isOutput=False)
    w3 = nc.declare_dram_parameter("w3", [2, 2, 3, 128, 128], F32, isOutput=False)
    w4 = nc.declare_dram_parameter("w4", [3, 128, 128], F32, isOutput=False)
    idn = nc.declare_dram_parameter("ident", [128, 128], F32, isOutput=False)
    out = nc.declare_dram_parameter("out", [2, 1 << 21], F32, isOutput=True)

    # AllToAll bounce buffers: [block(dest/src rank), plane, part, inner]
    a2a_in = nc.dram_tensor("a2a_in", [8, 2, 128, 2048], SD_DRAM)
    a2a_out = nc.dram_tensor("a2a_out", [8, 2, 128, 2048], SD_DRAM)

    SD = F32R if use_f32r else F32  # SBUF/bounce storage dtype

    def ldma(out_ap, in_ap):
        # DRAM f32 -> SBUF f32r needs the SWDGE cast path
        if use_f32r:
            nc.gpsimd.dma_start(out=out_ap, in_=in_ap)
        else:
            nc.sync.dma_start(out=out_ap, in_=in_ap)

    with tile.TileContext(nc, num_cores=N_CORES) as tc:
        with tc.tile_pool(name="state", bufs=1) as sp, \
             tc.tile_pool(name="wpool", bufs=1) as wp, \
             tc.tile_pool(name="mm", bufs=6, space="PSUM") as mmp, \
             tc.tile_pool(name="tr", bufs=2, space="PSUM") as trp:

            sre = sp.tile([128, 16384], SD, tag="sre")
            sim = sp.tile([128, 16384], SD, tag="sim")

            # ---- load weights ----
            def load_w3(dram_ap3, name):  # (3,128,128) -> 3 sbuf tiles
                ts = []
                for i in range(3):
                    t = wp.tile([128, 128], SD, tag=f"{name}_{i}")
                    ldma(t[:], dram_ap3[i])
                    ts.append(t)
                return ts

            w1t = load_w3(w1, "w1")
            w4t = load_w3(w4, "w4")
            w2t = [[load_w3(w2[j, k], f"w2_{j}{k}") for k in (0, 1)] for j in (0, 1)]
            w3t = [[load_w3(w3[j, k], f"w3_{j}{k}") for k in (0, 1)] for j in (0, 1)]
            idt = wp.tile([128, 128], SD, tag="ident")
            ldma(idt[:], idn[:])

            # ---- load state:  partitions (q0..q6), free (q7..q20) ----
            for pl, s in ((0, sre), (1, sim)):
                ldma(s[:], st[pl].rearrange("(p f) -> p f", p=128))

            planes = ((sre, sim))

            def cmul_into(pre, pim, W, xre, xim, start, stop=False):
                """pre += re(U)@xre - im(U)@xim ; pim += im(U)@xre + re(U)@xim
                W = [A, B, Bn] lhsT tiles."""
                A, B, Bn = W
                nc.tensor.matmul(pre[:], (A[:]), (xre), start=start, stop=False)
                nc.tensor.matmul(pre[:], (Bn[:]), (xim), start=False, stop=stop)
                nc.tensor.matmul(pim[:], (B[:]), (xre), start=start, stop=False)
                nc.tensor.matmul(pim[:], (A[:]), (xim), start=False, stop=stop)

            # ---- P1: chunk on partitions (q0..q6) ----
            for t in range(32):
                c0 = t * 512
                pre = mmp.tile([128, 512], F32, tag="mm")
                pim = mmp.tile([128, 512], F32, tag="mm")
                xre = sre[:, c0:c0 + 512]
                xim = sim[:, c0:c0 + 512]
                A, B, Bn = w1t
                nc.tensor.matmul(pre[:], (A[:]), (xre), start=True, stop=False)
                nc.tensor.matmul(pre[:], (Bn[:]), (xim), start=False, stop=True)
                nc.tensor.matmul(pim[:], (B[:]), (xre), start=True, stop=False)
                nc.tensor.matmul(pim[:], (A[:]), (xim), start=False, stop=True)
                nc.vector.tensor_copy(sre[:, c0:c0 + 512], pre[:])
                nc.scalar.copy(out=sim[:, c0:c0 + 512], in_=pim[:])

            # ---- T1: transpose partitions (q0..q6) <-> free (q7..q13) ----
            # L1 free = (q7..q13)*128 + (q14..q20); window w = (q14..q20):
            # read col-set {a*128+w}, transpose, write back to same col-set,
            # giving L2: partitions (q7..q13), free = (q0..q6)*128 + (q14..q20).
            for s in (sre, sim):
                sv = s[:].rearrange("p (a w) -> p a w", w=128)
                for w in range(128):
                    pt = trp.tile([128, 128], SD, tag="tr")
                    nc.tensor.transpose((pt[:]), (sv[:, :, w]), (idt[:]))
                    nc.vector.tensor_copy(sv[:, :, w], pt[:])

            # ---- P2: chunk [6..13]; partitions (q7..q13), q6 = free bit ----
            # L2 free = (q0..q6)*128 + (q14..q20); q6 = bit0 of the outer
            # index => columns alternate 128-blocks by q6.
            sre_v = sre[:].rearrange("p (o q c) -> p o q c", q=2, c=128)
            sim_v = sim[:].rearrange("p (o q c) -> p o q c", q=2, c=128)
            for t in range(16):
                o0 = t * 4
                xr = [sre_v[:, o0:o0 + 4, k, :] for k in (0, 1)]
                xi = [sim_v[:, o0:o0 + 4, k, :] for k in (0, 1)]
                ps = []
                for j in (0, 1):
                    pre = mmp.tile([128, 512], F32, tag="mm")
                    pim = mmp.tile([128, 512], F32, tag="mm")
                    cmul_into(pre, pim, w2t[j][0], xr[0], xi[0], start=True)
                    cmul_into(pre, pim, w2t[j][1], xr[1], xi[1], start=False, stop=True)
                    ps.append((pre, pim))
                for j in (0, 1):
                    pre, pim = ps[j]
                    nc.vector.tensor_copy(sre_v[:, o0:o0 + 4, j, :], pre[:])
                    nc.scalar.copy(out=sim_v[:, o0:o0 + 4, j, :], in_=pim[:])

            # ---- T2: transpose partitions (q7..q13) <-> free (q14..q20) ----
            # window o = (q0..q6): read contiguous block [o*128, o*128+128),
            # write back contiguous, giving L3: partitions (q14..q20),
            # free = (q0..q6)*128 + (q7..q13) = (q0..q13) natural.
            for s in (sre, sim):
                for o in range(128):
                    pt = trp.tile([128, 128], SD, tag="tr")
                    nc.tensor.transpose(
                        (pt[:]), (s[:, o * 128:o * 128 + 128]), (idt[:]))
                    nc.vector.tensor_copy(s[:, o * 128:o * 128 + 128], pt[:])

            # ---- P3: chunk [13..20]; partitions (q14..q20), q13 = free bit0 ----
            sre_w = sre[:].rearrange("p (c k) -> p c k", k=2)
            sim_w = sim[:].rearrange("p (c k) -> p c k", k=2)
            for t in range(16):
                c0 = t * 512
                xr = [sre_w[:, c0:c0 + 512, k] for k in (0, 1)]
                xi = [sim_w[:, c0:c0 + 512, k] for k in (0, 1)]
                ps = []
                for j in (0, 1):
                    pre = mmp.tile([128, 512], F32, tag="mm")
                    pim = mmp.tile([128, 512], F32, tag="mm")
                    cmul_into(pre, pim, w3t[j][0], xr[0], xi[0], start=True)
                    cmul_into(pre, pim, w3t[j][1], xr[1], xi[1], start=False, stop=True)
                    ps.append((pre, pim))
                for j in (0, 1):
                    pre, pim = ps[j]
                    nc.vector.tensor_copy(sre_w[:, c0:c0 + 512, j], pre[:])
                    nc.scalar.copy(out=sim_w[:, c0:c0 + 512, j], in_=pim[:])

            # ---- A2A staging:  SBUF (part q14..q20, free q0..q13) ->
            #      a2a_in[b = (q0,q1,q2), plane, part, (q3..q13)] ----
            for pl, s in ((0, sre), (1, sim)):
                nc.sync.dma_start(
                    out=a2a_in[:, pl].rearrange("b p f -> p b f"),
                    in_=s[:].rearrange("p (b f) -> p b f", b=8))

            nc.gpsimd.collective_compute(
                "AllToAll",
                mybir.AluOpType.bypass,
                replica_groups=[list(range(N_CORES))],
                ins=[a2a_in.ap().opt()],
                outs=[a2a_out.ap().opt()],
            )

            # ---- P4 readback: a2a_out[s3, pl, (h,m), f] ->
            #      partitions (s3,m) = s3*16+m, free = h*2048 + f,
            #      h = (q14,q15,q16), m = (q17..q20), f = (q3..q13) ----
            for s3 in range(8):
                for pl, s in ((0, sre), (1, sim)):
                    nc.sync.dma_start(
                        out=s[s3 * 16:(s3 + 1) * 16, :]
                            .rearrange("m (h f) -> m h f", h=8),
                        in_=a2a_out[s3, pl].rearrange("(h m) f -> m h f", m=16))

            # ---- P4: chunk [20..23] on partitions (q21,q22,q23,q17..q20) ----
            for t in range(32):
                c0 = t * 512
                pre = mmp.tile([128, 512], F32, tag="mm")
                pim = mmp.tile([128, 512], F32, tag="mm")
                xre = sre[:, c0:c0 + 512]
                xim = sim[:, c0:c0 + 512]
                A, B, Bn = w4t
                nc.tensor.matmul(pre[:], (A[:]), (xre), start=True, stop=False)
                nc.tensor.matmul(pre[:], (Bn[:]), (xim), start=False, stop=True)
                nc.tensor.matmul(pim[:], (B[:]), (xre), start=True, stop=False)
                nc.tensor.matmul(pim[:], (A[:]), (xim), start=False, stop=True)
                nc.vector.tensor_copy(sre[:, c0:c0 + 512], pre[:])
                nc.scalar.copy(out=sim[:, c0:c0 + 512], in_=pim[:])

            # ---- store:  out[pl] = partition-major flat ----
            for pl, s in ((0, sre), (1, sim)):
                nc.sync.dma_start(
                    out=out[pl].rearrange("(p f) -> p f", p=128),
                    in_=s[:].bitcast(F32) if use_f32r else s[:])

    return nc


# ---------------------------------------------------------------------------
# Host wrapper
# ---------------------------------------------------------------------------

TRACE = False          # set by test harnesses to capture a profile
LAST_EXEC_NS = None
LAST_RESULTS = None


def kernel(state, gates1, gates2):
    global LAST_EXEC_NS, LAST_RESULTS
    state = np.ascontiguousarray(np.asarray(state, dtype=np.float32))
    weights = build_weights(np.asarray(gates1, dtype=np.float32),
                            np.asarray(gates2, dtype=np.float32))

    # shard over (q21,q22,q23) = index mod 8
    shards = np.ascontiguousarray(
        state.reshape(2, 1 << 21, 8).transpose(2, 0, 1))

    nc = build_nc()
    if not nc.is_finalized():
        nc.finalize()
    in_maps = [dict(weights, state=shards[d]) for d in range(N_CORES)]
    res = run_bass_kernel_spmd(nc, in_maps, core_ids=list(range(N_CORES)),
                               trace=TRACE)
    LAST_EXEC_NS = res.exec_time_ns
    LAST_RESULTS = res

    # unshard: core d holds (q0,q1,q2) = d;
    # out layout = [plane][s=(q21..q23), m=(q17..q20)][h=(q14..q16)][f=(q3..q13)]
    full = np.empty((2, 8, 2048, 8, 16, 8), dtype=np.float32)
    for d in range(N_CORES):
        od = res.results[d]["out"].reshape(2, 8, 16, 8, 2048)
        full[:, d] = od.transpose(0, 4, 3, 2, 1)
    return full.reshape(2, 1 << 24)


if __name__ == "__main__":
    rng = np.random.default_rng(0)
    state = rng.standard_normal((2, 1 << 24)).astype(np.float32)
    g1 = rng.standard_normal((24, 2, 2, 2)).astype(np.float32)
    g2 = rng.standard_normal((23, 2, 4, 4)).astype(np.float32)
    out = kernel(state, g1, g2)
    print(out.shape, out.dtype)


# revision 12
# speedup vs baseline: 1.8424x; 1.2361x over previous
"""Trainium2 Bass kernel for nn_Circuit_19275813225041.

24-qubit state-vector simulation: one layer of single-qubit gates on every
qubit, then a ladder of two-qubit gates on neighboring pairs (q, q+1),
q = 0..22, on a 2^24 complex state stored as (2, 2^24) float32 (re, im).

Strategy (8 NeuronCores):
  - Qubit q <-> bit q of the state index, bit 0 = MSB.
  - Shard the state over the 3 LSB qubits (q21,q22,q23): core d holds
    amplitudes with index % 8 == d (state-vector slicing).
  - Gates are fused on the host into 4 big chunk matrices:
      U1: 128x128 on qubits [0..6]    (singles 0..6, ladder (0,1)..(5,6))
      U2: 256x256 on qubits [6..13]   (singles 7..13, ladder (6,7)..(12,13))
      U3: 256x256 on qubits [13..20]  (singles 14..20, ladder (13,14)..(19,20))
      U4: 128x128 on qubits [21,22,23,17..20] (singles 21..23, ladder
          (20,21),(21,22),(22,23), identity on q17..q19)
    (The reference's _apply_gate has a permutation quirk for the 2-qubit
    gate at q=1 -- its "inverse" transpose applies perm again, which for
    q=1 is a 3-cycle.  This adds a relabeling permutation on qubits
    (0,1,2) right after that gate; it is folded into U1.)
  - Each core applies U1..U3 to its local 2^21 state via TensorE matmuls,
    with PE transposes rotating 7-bit groups through the partition axis and
    2-term PSUM accumulation handling the chunk boundary bit (q6, q13).
  - One AllToAll swaps qubits (q0,q1,q2) <-> (q21,q22,q23) across cores so
    the final chunk U4 applies locally; output is returned sharded over
    (q0,q1,q2) and reassembled on the host.
"""

import numpy as np

import concourse.bass as bass
import concourse.bacc as bacc
import concourse.mybir as mybir
import concourse.tile as tile
from concourse.bass_utils import run_bass_kernel_spmd

F32 = mybir.dt.float32
F32R = mybir.dt.float32r

USE_F32R = True   # fast fp32 matmul mode (reduced precision)
N_CORES = 8


# ---------------------------------------------------------------------------
# Host-side gate fusion
# ---------------------------------------------------------------------------

def _embed_gate(mat, qubits, group):
    """Embed `mat` acting on `qubits` (MSB-first) into the space indexed by
    `group` (list of qubits, group[0] = MSB of the index)."""
    g = len(group)
    k = len(qubits)
    pos = [group.index(q) for q in qubits]
    rest = [i for i in range(g) if i not in pos]
    U = np.zeros((1 << g, 1 << g), dtype=np.complex128)
    for r in range(1 << len(rest)):
        base = 0
        for bi, p in enumerate(rest):
            if (r >> (len(rest) - 1 - bi)) & 1:
                base |= 1 << (g - 1 - p)
        for a in range(1 << k):
            ia = base
            for bi, p in enumerate(pos):
                if (a >> (k - 1 - bi)) & 1:
                    ia |= 1 << (g - 1 - p)
            for b in range(1 << k):
                ib = base
                for bi, p in enumerate(pos):
                    if (b >> (k - 1 - bi)) & 1:
                        ib |= 1 << (g - 1 - p)
                U[ia, ib] = mat[a, b]
    return U


def _quirk_P():
    # reference._apply_gate on [1,2]: the un-permute uses perm (a 3-cycle)
    # instead of its inverse => extra relabeling on qubits (0,1,2):
    # new (b0,b1,b2) = (old b2, old b0, old b1).
    P = np.zeros((8, 8), dtype=np.complex128)
    for b0 in range(2):
        for b1 in range(2):
            for b2 in range(2):
                P[(b2 << 2) | (b0 << 1) | b1, (b0 << 2) | (b1 << 1) | b2] = 1
    return P


def _fuse(ops, group):
    U = np.eye(1 << len(group), dtype=np.complex128)
    for mat, qb in ops:
        U = _embed_gate(mat, qb, group) @ U
    return U


def build_chunk_matrices(gates1, gates2):
    g1 = gates1[:, 0].astype(np.float64) + 1j * gates1[:, 1].astype(np.float64)
    g2 = gates2[:, 0].astype(np.float64) + 1j * gates2[:, 1].astype(np.float64)

    ops1 = [(g1[q], [q]) for q in range(0, 7)]
    ops1 += [(g2[0], [0, 1]), (g2[1], [1, 2]), (_quirk_P(), [0, 1, 2])]
    ops1 += [(g2[q], [q, q + 1]) for q in range(2, 6)]
    U1 = _fuse(ops1, list(range(0, 7)))

    ops2 = [(g1[q], [q]) for q in range(7, 14)]
    ops2 += [(g2[q], [q, q + 1]) for q in range(6, 13)]
    U2 = _fuse(ops2, list(range(6, 14)))  # q6 = MSB of the 256 index

    ops3 = [(g1[q], [q]) for q in range(14, 21)]
    ops3 += [(g2[q], [q, q + 1]) for q in range(13, 20)]
    U3 = _fuse(ops3, list(range(13, 21)))  # q13 = MSB

    ops4 = [(g1[q], [q]) for q in range(21, 24)]
    ops4 += [(g2[q], [q, q + 1]) for q in range(20, 23)]
    # partition index on the device = s*16 + m, s = (q21,q22,q23), m = (q17..q20)
    U4 = _fuse(ops4, [21, 22, 23, 17, 18, 19, 20])

    return U1, U2, U3, U4


def _pack_lhsT(U):
    """lhsT components for out = U @ x (complex):  A = re(U)^T, B = im(U)^T,
    Bn = -im(U)^T, stacked (3, n, n) float32."""
    return np.stack([U.real.T, U.imag.T, -U.imag.T]).astype(np.float32)


def build_weights(gates1, gates2):
    U1, U2, U3, U4 = build_chunk_matrices(gates1, gates2)
    w1 = _pack_lhsT(U1)
    w4 = _pack_lhsT(U4)

    def blocks(U):  # (2, 2, 3, 128, 128)
        return np.stack([
            np.stack([_pack_lhsT(U[j * 128:(j + 1) * 128, k * 128:(k + 1) * 128])
                      for k in (0, 1)])
            for j in (0, 1)])

    w2 = blocks(U2)
    w3 = blocks(U3)
    ident = np.eye(128, dtype=np.float32)
    return {"w1": w1, "w2": w2, "w3": w3, "w4": w4, "ident": ident}


# ---------------------------------------------------------------------------
# Bass kernel builder
# ---------------------------------------------------------------------------

def build_nc(use_f32r=USE_F32R):
    nc = bacc.Bacc()
    SD_DRAM = F32R if use_f32r else F32

    st = nc.declare_dram_parameter("state", [2, 1 << 21], F32, isOutput=False)
    w1 = nc.declare_dram_parameter("w1", [3, 128, 128], F32, isOutput=False)
    w2 = nc.declare_dram_parameter("w2", [2, 2, 3, 128, 128], F32, isOutput=False)
    w3 = nc.declare_dram_parameter("w3", [2, 2, 3, 128, 128], F32, isOutput=False)
    w4 = nc.declare_dram_parameter("w4", [3, 128, 128], F32, isOutput=False)
    idn = nc.declare_dram_parameter("ident", [128, 128], F32, isOutput=False)
    out = nc.declare_dram_parameter("out", [2, 1 << 21], F32, isOutput=True)

    # AllToAll bounce buffers, split in two halves by q3 so the collective
    # overlaps P3/P4 compute: [block(dest/src rank), plane, part, inner/2]
    a2a_in = [nc.dram_tensor(f"a2a_in{h}", [8, 2, 128, 1024], SD_DRAM)
              for h in (0, 1)]
    a2a_out = [nc.dram_tensor(f"a2a_out{h}", [8, 2, 128, 1024], SD_DRAM)
               for h in (0, 1)]

    SD = F32R if use_f32r else F32  # SBUF/bounce storage dtype

    def ldma(out_ap, in_ap):
        # DRAM f32 -> SBUF f32r needs the SWDGE cast path
        if use_f32r:
            nc.gpsimd.dma_start(out=out_ap, in_=in_ap)
        else:
            nc.sync.dma_start(out=out_ap, in_=in_ap)

    with tile.TileContext(nc, num_cores=N_CORES) as tc:
        with tc.tile_pool(name="state", bufs=1) as sp, \
             tc.tile_pool(name="wpool", bufs=1) as wp, \
             tc.tile_pool(name="mm", bufs=6, space="PSUM") as mmp, \
             tc.tile_pool(name="tr", bufs=2, space="PSUM") as trp:

            sre = sp.tile([128, 16384], SD, tag="sre")
            sim = sp.tile([128, 16384], SD, tag="sim")

            # ---- load weights ----
            def load_w3(dram_ap3, name):  # (3,128,128) -> 3 sbuf tiles
                ts = []
                for i in range(3):
                    t = wp.tile([128, 128], SD, tag=f"{name}_{i}")
                    ldma(t[:], dram_ap3[i])
                    ts.append(t)
                return ts

            w1t = load_w3(w1, "w1")
            w4t = load_w3(w4, "w4")
            w2t = [[load_w3(w2[j, k], f"w2_{j}{k}") for k in (0, 1)] for j in (0, 1)]
            w3t = [[load_w3(w3[j, k], f"w3_{j}{k}") for k in (0, 1)] for j in (0, 1)]
            idt = wp.tile([128, 128], SD, tag="ident")
            ldma(idt[:], idn[:])

            # ---- load state:  partitions (q0..q6), free (q7..q20) ----
            st_v = [st[pl].rearrange("(p f) -> p f", p=128) for pl in (0, 1)]
            for c in range(4):
                for pl, s in ((0, sre), (1, sim)):
                    ldma(s[:, c * 4096:(c + 1) * 4096],
                         st_v[pl][:, c * 4096:(c + 1) * 4096])

            planes = ((sre, sim))

            def cmul_into(pre, pim, W, xre, xim, start, stop=False):
                """pre += re(U)@xre - im(U)@xim ; pim += im(U)@xre + re(U)@xim
                W = [A, B, Bn] lhsT tiles."""
                A, B, Bn = W
                nc.tensor.matmul(pre[:], (A[:]), (xre), start=start, stop=False)
                nc.tensor.matmul(pre[:], (Bn[:]), (xim), start=False, stop=stop)
                nc.tensor.matmul(pim[:], (B[:]), (xre), start=start, stop=False)
                nc.tensor.matmul(pim[:], (A[:]), (xim), start=False, stop=stop)

            # ---- P1: chunk on partitions (q0..q6) ----
            for t in range(32):
                c0 = t * 512
                pre = mmp.tile([128, 512], F32, tag="mm")
                pim = mmp.tile([128, 512], F32, tag="mm")
                xre = sre[:, c0:c0 + 512]
                xim = sim[:, c0:c0 + 512]
                A, B, Bn = w1t
                nc.tensor.matmul(pre[:], (A[:]), (xre), start=True, stop=False)
                nc.tensor.matmul(pre[:], (Bn[:]), (xim), start=False, stop=True)
                nc.tensor.matmul(pim[:], (B[:]), (xre), start=True, stop=False)
                nc.tensor.matmul(pim[:], (A[:]), (xim), start=False, stop=True)
                nc.vector.tensor_copy(sre[:, c0:c0 + 512], pre[:])
                nc.scalar.copy(out=sim[:, c0:c0 + 512], in_=pim[:])

            # ---- T1: transpose partitions (q0..q6) <-> free (q7..q13) ----
            # L1 free = (q7..q13)*128 + (q14..q20); window w = (q14..q20):
            # read col-set {a*128+w}, transpose, write back to same col-set,
            # giving L2: partitions (q7..q13), free = (q0..q6)*128 + (q14..q20).
            for si, s in enumerate((sre, sim)):
                sv = s[:].rearrange("p (a w) -> p a w", w=128)
                for w in range(128):
                    pt = trp.tile([128, 128], SD, tag="tr")
                    nc.tensor.transpose((pt[:]), (sv[:, :, w]), (idt[:]))
                    if (w + si) % 3 == 0:
                        nc.scalar.copy(out=sv[:, :, w], in_=pt[:])
                    else:
                        nc.vector.tensor_copy(sv[:, :, w], pt[:])

            # ---- P2: chunk [6..13]; partitions (q7..q13), q6 = free bit ----
            # L2 free = (q0..q6)*128 + (q14..q20); q6 = bit0 of the outer
            # index => columns alternate 128-blocks by q6.
            sre_v = sre[:].rearrange("p (o q c) -> p o q c", q=2, c=128)
            sim_v = sim[:].rearrange("p (o q c) -> p o q c", q=2, c=128)
            for t in range(16):
                o0 = t * 4
                xr = [sre_v[:, o0:o0 + 4, k, :] for k in (0, 1)]
                xi = [sim_v[:, o0:o0 + 4, k, :] for k in (0, 1)]
                ps = []
                for j in (0, 1):
                    pre = mmp.tile([128, 512], F32, tag="mm")
                    pim = mmp.tile([128, 512], F32, tag="mm")
                    cmul_into(pre, pim, w2t[j][0], xr[0], xi[0], start=True)
                    cmul_into(pre, pim, w2t[j][1], xr[1], xi[1], start=False, stop=True)
                    ps.append((pre, pim))
                for j in (0, 1):
                    pre, pim = ps[j]
                    nc.vector.tensor_copy(sre_v[:, o0:o0 + 4, j, :], pre[:])
                    nc.scalar.copy(out=sim_v[:, o0:o0 + 4, j, :], in_=pim[:])

            # ---- T2: transpose partitions (q7..q13) <-> free (q14..q20) ----
            # window o = (q0..q6): read contiguous block [o*128, o*128+128),
            # write back contiguous, giving L3: partitions (q14..q20),
            # free = (q0..q6)*128 + (q7..q13) = (q0..q13) natural.
            for si, s in enumerate((sre, sim)):
                for o in range(128):
                    pt = trp.tile([128, 128], SD, tag="tr")
                    nc.tensor.transpose(
                        (pt[:]), (s[:, o * 128:o * 128 + 128]), (idt[:]))
                    if (o + si) % 3 == 0:
                        nc.scalar.copy(out=s[:, o * 128:o * 128 + 128], in_=pt[:])
                    else:
                        nc.vector.tensor_copy(s[:, o * 128:o * 128 + 128], pt[:])

            # ---- P3: chunk [13..20]; partitions (q14..q20), q13 = free bit0 ----
            sre_w = sre[:].rearrange("p (c k) -> p c k", k=2)
            sim_w = sim[:].rearrange("p (c k) -> p c k", k=2)

            def p3_tile(t):
                c0 = t * 512
                xr = [sre_w[:, c0:c0 + 512, k] for k in (0, 1)]
                xi = [sim_w[:, c0:c0 + 512, k] for k in (0, 1)]
                ps = []
                for j in (0, 1):
                    pre = mmp.tile([128, 512], F32, tag="mm")
                    pim = mmp.tile([128, 512], F32, tag="mm")
                    cmul_into(pre, pim, w3t[j][0], xr[0], xi[0], start=True)
                    cmul_into(pre, pim, w3t[j][1], xr[1], xi[1], start=False, stop=True)
                    ps.append((pre, pim))
                for j in (0, 1):
                    pre, pim = ps[j]
                    nc.vector.tensor_copy(sre_w[:, c0:c0 + 512, j], pre[:])
                    nc.scalar.copy(out=sim_w[:, c0:c0 + 512, j], in_=pim[:])

            def stage_half(h):
                # SBUF (part q14..q20, free (q0..q13)) ->
                # a2a_in[h][b=(q0q1q2), pl, part, (q4..q13)], q3 = h
                for pl, s in ((0, sre), (1, sim)):
                    sv = s[:].rearrange("p (b g f) -> p b g f", b=8, g=2)
                    nc.sync.dma_start(
                        out=a2a_in[h][:, pl].rearrange("b p f -> p b f"),
                        in_=sv[:, :, h, :])
                nc.gpsimd.collective_compute(
                    "AllToAll",
                    mybir.AluOpType.bypass,
                    replica_groups=[list(range(N_CORES))],
                    ins=[a2a_in[h].ap().opt()],
                    outs=[a2a_out[h].ap().opt()],
                )

            def readback_half(h):
                # a2a_out[h][s3, pl, (h3,m), f] -> partitions s3*16+m,
                # free = h3*2048 + h*1024 + f, f = (q4..q13)
                for s3 in range(8):
                    for pl, s in ((0, sre), (1, sim)):
                        sv = (s[s3 * 16:(s3 + 1) * 16, :]
                              .rearrange("m (h3 g f) -> m h3 g f", h3=8, g=2))
                        nc.sync.dma_start(
                            out=sv[:, :, h, :],
                            in_=a2a_out[h][s3, pl]
                                .rearrange("(h3 m) f -> m h3 f", m=16))

            for t in range(0, 16, 2):  # q3 = 0 tiles
                p3_tile(t)
            stage_half(0)
            for t in range(1, 16, 2):  # q3 = 1 tiles
                p3_tile(t)
            stage_half(1)
            readback_half(0)
            readback_half(1)

            # ---- P4: chunk [20..23] on partitions (q21,q22,q23,q17..q20) ----
            p4_order = [t for t in range(32) if (t >> 1) & 1 == 0] + \
                       [t for t in range(32) if (t >> 1) & 1 == 1]
            for t in p4_order:
                c0 = t * 512
                pre = mmp.tile([128, 512], F32, tag="mm")
                pim = mmp.tile([128, 512], F32, tag="mm")
                xre = sre[:, c0:c0 + 512]
                xim = sim[:, c0:c0 + 512]
                A, B, Bn = w4t
                nc.tensor.matmul(pre[:], (A[:]), (xre), start=True, stop=False)
                nc.tensor.matmul(pre[:], (Bn[:]), (xim), start=False, stop=True)
                nc.tensor.matmul(pim[:], (B[:]), (xre), start=True, stop=False)
                nc.tensor.matmul(pim[:], (A[:]), (xim), start=False, stop=True)
                nc.vector.tensor_copy(sre[:, c0:c0 + 512], pre[:])
                nc.scalar.copy(out=sim[:, c0:c0 + 512], in_=pim[:])
                # store this 512-col chunk as soon as it's final
                for pl, s in ((0, sre), (1, sim)):
                    ov = out[pl].rearrange("(p f) -> p f", p=128)
                    nc.sync.dma_start(
                        out=ov[:, c0:c0 + 512],
                        in_=(s[:, c0:c0 + 512].bitcast(F32)
                             if use_f32r else s[:, c0:c0 + 512]))

    return nc


# ---------------------------------------------------------------------------
# Host wrapper
# ---------------------------------------------------------------------------

TRACE = False          # set by test harnesses to capture a profile
LAST_EXEC_NS = None
LAST_RESULTS = None


def kernel(state, gates1, gates2):
    global LAST_EXEC_NS, LAST_RESULTS
    state = np.ascontiguousarray(np.asarray(state, dtype=np.float32))
    weights = build_weights(np.asarray(gates1, dtype=np.float32),
                            np.asarray(gates2, dtype=np.float32))

    # shard over (q21,q22,q23) = index mod 8
    shards = np.ascontiguousarray(
        state.reshape(2, 1 << 21, 8).transpose(2, 0, 1))

    nc = build_nc()
    if not nc.is_finalized():
        nc.finalize()
    in_maps = [dict(weights, state=shards[d]) for d in range(N_CORES)]
    res = run_bass_kernel_spmd(nc, in_maps, core_ids=list(range(N_CORES)),
                               trace=TRACE)
    LAST_EXEC_NS = res.exec_time_ns
    LAST_RESULTS = res

    # unshard: core d holds (q0,q1,q2) = d;
    # out layout = [plane][s=(q21..q23), m=(q17..q20)][h=(q14..q16)][f=(q3..q13)]
    full = np.empty((2, 8, 2048, 8, 16, 8), dtype=np.float32)
    for d in range(N_CORES):
        od = res.results[d]["out"].reshape(2, 8, 16, 8, 2048)
        full[:, d] = od.transpose(0, 4, 3, 2, 1)
    return full.reshape(2, 1 << 24)


if __name__ == "__main__":
    rng = np.random.default_rng(0)
    state = rng.standard_normal((2, 1 << 24)).astype(np.float32)
    g1 = rng.standard_normal((24, 2, 2, 2)).astype(np.float32)
    g2 = rng.standard_normal((23, 2, 4, 4)).astype(np.float32)
    out = kernel(state, g1, g2)
    print(out.shape, out.dtype)
